# revision 3
# baseline (speedup 1.0000x reference)
"""Trainium2 Bass kernel v2 for windowed local attention (8x8 windows).

Math (per core = one image [C=192, 256, 256], weights replicated):
  window partition (8x8) -> per window: qkv = Wqkv x + b; A = softmax(q^T k /
  sqrt(C)); out = v A^T; y = Wp out + bp -> window reverse.

Key algebraic restructure vs v1:
  y = Wp (v A^T) + bp = (Wp Wv x + Wp bv) A^T + bp  (A^T columns sum to 1)
    = (Wu x) A^T + bp'   with Wu = Wp Wv, bp' = bp + Wp bv.
  The proj stage disappears; "u" is produced TRANSPOSED ([tokens, chan])
  directly from a window-major copy of x by swapping matmul operands:
    uT[t, c] = sum_c' xw[c', t] WuT[c', c]   (lhsT = xw slice, rhs = WuT).

Speed levers vs v1 (which ran all matmuls in fp32 = 4 cyc/row):
  - band q,k matmuls in float32r (1 cyc/row at N>=256); attention-side
    matmuls in bf16 (1 cyc/row at any N).
  - q2/k2 packed into one 128-row PSUM block [q2;k2]; k2 realigned to
    partition base 0 with one SBUF->SBUF DMA per band.
  - window-major bf16 copy of x (xw) built on the otherwise-idle GPSIMD
    engine; feeds the uT matmuls.
  - softmax per group of 4 window pairs: one exp on ACT, cross-window
    garbage zeroed by GPSIMD memsets, row-sum + reciprocal + per-row
    scale on DVE.
  - software pipelining: per iteration (= one group) the PE runs
    y(i-3), attnT(i-2), band-matmul chunk of band+1, uT(i), scores(i);
    ACT runs out-copies(i-3), q/k copies, exp(i) last; DVE runs
    ats-copy(i-2), qk2-copy, uts-copy(i), softmax tail(i). All PSUM->SBUF
    copies hide under neighboring-iteration PE work.

Sharding: data-parallel over batch; core b handles image b.
"""

import sys

import numpy as np

if "/opt/trn_rl_repo" not in sys.path:
    sys.path.insert(0, "/opt/trn_rl_repo")

C = 192
WS = 8
S = WS * WS  # 64 tokens per window

LAG_AT = 3  # attnT transpose lag (iterations)
LAG_Y = 4   # y matmul + out-copy lag


def build_program(n_bands=32, width=256):
    import concourse.bass as bass  # noqa: F401
    import concourse.tile as tile
    from concourse import bacc, mybir

    f32 = mybir.dt.float32
    f32r = mybir.dt.float32r
    bf16 = mybir.dt.bfloat16

    NW = width // WS          # 32 windows per band
    NG = NW // 8              # 4 groups per band (4 pairs = 8 windows each)
    NT = WS * width           # 2048 tokens per band
    NCK = NT // 512           # 4 psum chunks per band

    nc = bacc.Bacc("TRN2", target_bir_lowering=False, debug=False)

    Hn = n_bands * WS
    x = nc.dram_tensor("x", [C, Hn, width], f32r, kind="ExternalInput").ap()
    y = nc.dram_tensor("y", [C, Hn, width], f32, kind="ExternalOutput").ap()
    # packed constants: few DMAs (each dma_start costs ~625ns on HWDGE)
    wca = nc.dram_tensor("wca", [128, 384], f32r, kind="ExternalInput").ap()
    wcb = nc.dram_tensor("wcb", [64, 384], f32r, kind="ExternalInput").ap()
    bfca = nc.dram_tensor("bfca", [128, C + 128], bf16, kind="ExternalInput").ap()
    bfcb = nc.dram_tensor("bfcb", [64, C], bf16, kind="ExternalInput").ap()
    bca = nc.dram_tensor("bca", [128, 4], f32, kind="ExternalInput").ap()
    bcb = nc.dram_tensor("bcb", [64, 1], f32, kind="ExternalInput").ap()

    Ident = mybir.ActivationFunctionType.Identity
    Exp = mybir.ActivationFunctionType.Exp
    AX = mybir.AxisListType.X

    from contextlib import ExitStack

    with tile.TileContext(nc) as tc, ExitStack() as ctx:
        cp = ctx.enter_context(tc.tile_pool(name="consts", bufs=1))
        xp = ctx.enter_context(tc.tile_pool(name="xbands", bufs=3))
        xwp = ctx.enter_context(tc.tile_pool(name="xw", bufs=2))
        qkp = ctx.enter_context(tc.tile_pool(name="qk", bufs=2))
        utsp = ctx.enter_context(tc.tile_pool(name="uts", bufs=6))
        ep = ctx.enter_context(tc.tile_pool(name="e", bufs=5))
        rp = ctx.enter_context(tc.tile_pool(name="r", bufs=5))
        atsp = ctx.enter_context(tc.tile_pool(name="ats", bufs=2))
        fbp = ctx.enter_context(tc.tile_pool(name="fb", bufs=2))
        pp_bm = ctx.enter_context(tc.tile_pool(name="pp_bm", bufs=3, space="PSUM"))
        pp_ut = ctx.enter_context(tc.tile_pool(name="pp_ut", bufs=1, space="PSUM"))
        pp_ef = ctx.enter_context(tc.tile_pool(name="pp_ef", bufs=3, space="PSUM"))

        band_state = {}

        def dma_in(hw, chunked=False):
            x1 = xp.tile([128, WS, width], f32r, tag="x1")
            x2 = xp.tile([64, WS, width], f32r, tag="x2")
            nck = 4 if chunked else 1
            rw = WS // nck
            for c2 in range(nck):
                r0, r1 = hw * WS + c2 * rw, hw * WS + (c2 + 1) * rw
                nc.sync.dma_start(out=x1[:, c2 * rw:(c2 + 1) * rw],
                                  in_=x[0:128, r0:r1, :])
                nc.sync.dma_start(out=x2[:, c2 * rw:(c2 + 1) * rw],
                                  in_=x[128:C, r0:r1, :])
            band_state[hw] = dict(x1=x1, x2=x2)

        dma_in(0)
        if n_bands > 1:
            dma_in(1)

        def const_2d(name, src, p, cols, dt):
            t = cp.tile([p, cols], dt, tag=name)
            nc.sync.dma_start(out=t[:], in_=src[0:p, 0:cols])
            return t

        wca_t = const_2d("wca", wca, 128, 384, f32r)
        wcb_t = const_2d("wcb", wcb, 64, 384, f32r)
        bfca_t = const_2d("bfca", bfca, 128, C + 128, bf16)
        bfcb_t = const_2d("bfcb", bfcb, 64, C, bf16)
        bca_t = const_2d("bca", bca, 128, 4, f32)
        bcb_t = const_2d("bcb", bcb, 64, 1, f32)
        wq1a_t = wca_t[:, 0:128]
        wk1a_t = wca_t[:, 128:256]
        wqk2a_t = wca_t[:, 256:384]
        wq1b_t = wcb_t[:, 0:128]
        wk1b_t = wcb_t[:, 128:256]
        wqk2b_t = wcb_t[:, 256:384]
        wua_t = bfca_t[:, 0:C]
        eye_t = bfca_t[:, C:C + 128]
        wub_t = bfcb_t[:, 0:C]
        bq1_t = bca_t[:, 0:1]
        bk1_t = bca_t[:, 1:2]
        bqk2_t = bca_t[:, 2:3]
        bp1_t = bca_t[:, 3:4]
        bp2_t = bcb_t[:, 0:1]

        iter_state = {}

        # persistent attnT tiles: garbage quadrants stay zero forever; the
        # per-iteration copies write only the valid 64x64 blocks
        ats_bufs = []
        for ai in range(2):
            abuf = atsp.tile([128, 4, 128], bf16, tag="ats", name=f"ats{ai}")
            nc.gpsimd.memset(abuf[:], 0.0)
            ats_bufs.append(abuf)

        def band_alloc(hw):
            st = band_state[hw]
            st["q1"] = qkp.tile([128, NT], bf16, tag="q1", name="q1")
            st["k1"] = qkp.tile([128, NT], bf16, tag="k1", name="k1")
            st["qk2"] = qkp.tile([128, NT], bf16, tag="qk2", name="qk2")
            st["k2s"] = qkp.tile([64, NT], bf16, tag="k2s", name="k2s")
            st["xw1"] = xwp.tile([128, NT], bf16, tag="xw1", name="xw1")
            st["xw2"] = xwp.tile([64, NT], bf16, tag="xw2", name="xw2")
            st["fb1"] = fbp.tile([128, WS, width], f32, tag="fb1", name="fb1")
            st["fb2"] = fbp.tile([64, WS, width], f32, tag="fb2", name="fb2")

        def band_chunk(hw, ck):
            """One 512-token chunk of band hw's q/k matmuls + copies, plus
            the window-major bf16 x copy for group ck (on GPSIMD)."""
            st = band_state[hw]
            x1, x2 = st["x1"], st["x2"]
            xf1 = x1[:].rearrange("p i w -> p (i w)")
            xf2 = x2[:].rearrange("p i w -> p (i w)")
            ri2 = 512 // width  # band rows per chunk
            rhs1 = xf1[:, ck * 512:(ck + 1) * 512]
            rhs2 = xf2[:, ck * 512:(ck + 1) * 512]
            for wa, wb, name, bias, eng in (
                (wq1a_t, wq1b_t, "q1", bq1_t, "act"),
                (wk1a_t, wk1b_t, "k1", bk1_t, "act"),
                (wqk2a_t, wqk2b_t, "qk2", bqk2_t, "dve"),
            ):
                sb = st[name]
                ps = pp_bm.tile([128, 512], f32, tag="bm")
                nc.tensor.matmul(ps[:], wa, rhs1, start=True, stop=False)
                nc.tensor.matmul(ps[:], wb, rhs2, start=False, stop=True)
                # window-major out view: chunk ck = band rows 2ck..2ck+2
                outv = sb[:].rearrange(
                    "p (ww i j) -> p i ww j", ww=NW, i=WS, j=WS
                )[:, ck * ri2:(ck + 1) * ri2]
                if eng == "act":
                    nc.scalar.activation(outv, ps[:], Ident, bias=bias)
                else:
                    nc.vector.tensor_scalar_add(outv, ps[:], bias)
            # window-major bf16 x for group ck (tokens ck*512..): GPSIMD
            for xs, xw in ((x1, st["xw1"]), (x2, st["xw2"])):
                inv = xs[:, :, 64 * ck:64 * ck + 64].rearrange(
                    "p i (ww j) -> p ww i j", ww=8, j=WS)
                outw = xw[:].rearrange("p (g t) -> p g t", g=NCK)[:, ck]
                nc.gpsimd.tensor_copy(outw, inv)
            if ck == NCK - 1:
                # realign k2 (= qk2 partitions 64:128) to partition base 0
                nc.sync.dma_start(out=st["k2s"][:], in_=st["qk2"][64:128, :])

        def stage0(it):
            """uT matmuls + copy; scores matmuls; softmax chain."""
            hw, g = divmod(it, NG)
            st = band_state[hw]
            q1, k1, qk2, k2s = st["q1"], st["k1"], st["qk2"], st["k2s"]
            xw1, xw2 = st["xw1"], st["xw2"]

            # --- uT: [tokens, chan] per pair, from window-major bf16 x ---
            # pair stride 256 so each matmul output stays inside one
            # 2KB PSUM bank half (192-wide at stride 192 would cross the
            # bank boundary for pair 2 and corrupt it)
            utp = pp_ut.tile([128, 4, 256], f32, tag="ut")
            for p in range(4):
                off = (g * 4 + p) * 128
                nc.tensor.matmul(utp[:, p, 0:C], xw1[:, off:off + 128], wua_t,
                                 start=True, stop=False)
                nc.tensor.matmul(utp[:, p, 0:C], xw2[:, off:off + 128], wub_t,
                                 start=False, stop=True)
            uts = utsp.tile([128, 4, C], bf16, tag="uts")
            nc.vector.tensor_copy(uts[:], utp[:, :, 0:C])

            # --- scores (pair-blocked; off-diagonal 64x64 blocks garbage) ---
            scp = pp_ef.tile([128, 4, 128], f32, tag="ef")
            for p in range(4):
                off = (g * 4 + p) * 128
                nc.tensor.matmul(scp[:, p], q1[:, off:off + 128],
                                 k1[:, off:off + 128], start=True, stop=False)
                nc.tensor.matmul(scp[:, p], qk2[0:64, off:off + 128],
                                 k2s[0:64, off:off + 128],
                                 start=False, stop=True)

            # --- softmax: exp -> zero garbage -> row-sum -> 1/x -> scale ---
            e = ep.tile([128, 4, 128], bf16, tag="e")
            nc.scalar.activation(e[:], scp[:], Exp)
            rs = rp.tile([128, 4], f32, tag="rs")
            nc.vector.reduce_sum(rs[0:64, :], e[0:64, :, 0:64], axis=AX)
            nc.vector.reduce_sum(rs[64:128, :], e[64:128, :, 64:128], axis=AX)
            ri = rp.tile([128, 4], f32, tag="ri")
            nc.vector.reciprocal(ri[:], rs[:])
            for p in range(4):
                nc.vector.tensor_scalar_mul(e[:, p, :], e[:, p, :],
                                            ri[:, p:p + 1])
            iter_state[it] = dict(uts=uts, e=e)

        def stage_at(it):
            """attnT = e^T via identity matmul (garbage blocks are zero)."""
            ist = iter_state[it]
            e = ist["e"]
            atp = pp_ef.tile([128, 4, 128], bf16, tag="ef")
            for p in range(4):
                nc.tensor.transpose(atp[:, p], e[:, p, :], eye_t)
            ats = ats_bufs[it % 2]
            nc.vector.tensor_copy(ats[0:64, :, 0:64], atp[0:64, :, 0:64])
            nc.vector.tensor_copy(ats[64:128, :, 64:128], atp[64:128, :, 64:128])
            ist["ats"] = ats

        def stage_y(it):
            """y = uT^T @ attnT per pair; biased copy into band out buffer."""
            hw, g = divmod(it, NG)
            ist = iter_state.pop(it)
            uts, ats = ist["uts"], ist["ats"]
            st = band_state[hw]
            y1 = pp_ef.tile([128, 4, 128], f32, tag="ef")
            y2 = pp_ef.tile([64, 4, 128], f32, tag="ef")
            for p in range(4):
                nc.tensor.matmul(y1[:, p], uts[:, p, 0:128], ats[:, p, :])
                nc.tensor.matmul(y2[:, p], uts[:, p, 128:C], ats[:, p, :])
            fr1 = st["fb1"][:].rearrange(
                "p i (gg w8 j) -> p gg w8 i j", gg=NG, w8=8, j=WS)
            fr2 = st["fb2"][:].rearrange(
                "p i (gg w8 j) -> p gg w8 i j", gg=NG, w8=8, j=WS)
            nc.scalar.activation(fr1[:, g], y1[:], Ident, bias=bp1_t)
            nc.scalar.activation(fr2[:, g], y2[:], Ident, bias=bp2_t)

        def dma_out(hw):
            st = band_state.pop(hw)
            nc.sync.dma_start(out=y[0:128, hw * WS:(hw + 1) * WS, :],
                              in_=st["fb1"][:])
            nc.sync.dma_start(out=y[128:C, hw * WS:(hw + 1) * WS, :],
                              in_=st["fb2"][:])

        total = n_bands * NG
        # prologue: band 0 fully prepared up-front (x DMAs already issued)
        band_alloc(0)
        for ck in range(NCK):
            band_chunk(0, ck)

        for it in range(total + LAG_Y):
            hw, g = divmod(it, NG)
            # lagged PE-first stages: their PSUM->SBUF copies + out-copies
            # land early in each engine's queue
            if LAG_Y <= it:
                stage_y(it - LAG_Y)
            if LAG_AT <= it < total + LAG_AT:
                stage_at(it - LAG_AT)
            if it < total:
                if hw + 1 < n_bands:
                    if g == 0:
                        band_alloc(hw + 1)
                        if hw + 2 < n_bands:
                            dma_in(hw + 2)
                    band_chunk(hw + 1, g)
                stage0(it)
            if LAG_Y <= it and (it - LAG_Y) % NG == NG - 1:
                dma_out((it - LAG_Y) // NG)

    nc.compile()
    return nc


def prep_weights(w_qkv, b_qkv, w_proj, b_proj):
    scale = np.float64(C) ** -0.5
    w_qkv = np.asarray(w_qkv, dtype=np.float64)
    b_qkv = np.asarray(b_qkv, dtype=np.float64)
    w_proj = np.asarray(w_proj, dtype=np.float64)
    b_proj = np.asarray(b_proj, dtype=np.float64)
    wq, wk, wv = w_qkv[0:C], w_qkv[C:2 * C], w_qkv[2 * C:3 * C]
    bq, bk, bv = b_qkv[0:C], b_qkv[C:2 * C], b_qkv[2 * C:3 * C]
    wqT = (wq * scale).T  # [c_in, c_out]
    wkT = wk.T
    wuT = (w_proj @ wv).T  # [c_in, c_out]
    bqs = bq * scale
    bp = b_proj + w_proj @ bv

    import ml_dtypes
    f = lambda a: np.ascontiguousarray(a, dtype=np.float32)
    b = lambda a: np.ascontiguousarray(a, dtype=ml_dtypes.bfloat16)
    wqk2a = np.concatenate([wqT[0:128, 128:C], wkT[0:128, 128:C]], axis=1)
    wqk2b = np.concatenate([wqT[128:C, 128:C], wkT[128:C, 128:C]], axis=1)
    bqk2 = np.concatenate([bqs[128:C], bk[128:C]])
    return {
        "wca": f(np.concatenate(
            [wqT[0:128, 0:128], wkT[0:128, 0:128], wqk2a], axis=1)),
        "wcb": f(np.concatenate(
            [wqT[128:C, 0:128], wkT[128:C, 0:128], wqk2b], axis=1)),
        "bfca": b(np.concatenate(
            [wuT[0:128], np.eye(128)], axis=1)),
        "bfcb": b(wuT[128:C]),
        "bca": f(np.stack(
            [bqs[0:128], bk[0:128], bqk2, bp[0:128]], axis=1)),
        "bcb": f(bp[128:C].reshape(64, 1)),
    }


_PROGRAM_CACHE = {}


def get_program(n_bands, width=256):
    key = (n_bands, width)
    if key not in _PROGRAM_CACHE:
        _PROGRAM_CACHE[key] = build_program(n_bands, width)
    return _PROGRAM_CACHE[key]


def make_in_maps(x, w_qkv, b_qkv, w_proj, b_proj):
    x = np.asarray(x, dtype=np.float32)
    wts = prep_weights(w_qkv, b_qkv, w_proj, b_proj)
    return [{"x": np.ascontiguousarray(x[b]), **wts} for b in range(x.shape[0])]


def kernel(x, w_qkv, b_qkv, w_proj, b_proj):
    from concourse.bass_utils import run_bass_kernel_spmd

    x = np.asarray(x, dtype=np.float32)
    B, c, H, W = x.shape
    assert c == C
    nc = get_program(H // WS, W)
    in_maps = make_in_maps(x, w_qkv, b_qkv, w_proj, b_proj)
    res = run_bass_kernel_spmd(nc, in_maps, core_ids=list(range(B)))
    out = np.stack([res.results[b]["y"] for b in range(B)], axis=0)
    return out.astype(np.float32)
